# revision 6
# baseline (speedup 1.0000x reference)
"""AttentionHead kernel for 8x TRN2 NeuronCores (Bass/Tile on Bacc).

Problem: single-head attention, S=4096, B=4, D=128, C=K=V=64, f32 inputs,
int32 {0,1} mask [1, S, S] applied before softmax (mask==0 -> -inf).

Sharding: queries sharded across 8 cores (512 q/core, all 4 batches per
core). The 64 MiB mask is read exactly once across the chip; key/value are
replicated (8 MiB each/core). Per-core HBM traffic ~25.5 MiB.

Math (per core, per batch), all PE contractions on partitions:
  scores^T[s, q] = sum_c k_proj[s,c] q_proj[q,c]      (lhsT = k_projT tile)
  alpha = exp(scores^T / 8) * maskT                    (ACT exp, DVE mult)
  va[d, q]   = sum_s value[s,d] alpha[s,q]             (value natural = lhsT)
  sums[q]    = sum_s alpha[s,q]                        (ones-column matmul)
  out^T_us   = wv @ va + bv (x) sums                   (rank-1 bias matmul)
  out[q, :]  = (out^T_us / sums).T                     (PE transpose + scale)

Key layout tricks:
  - s-tiles processed in even/odd pairs: even tile's k_projT lives on SBUF
    partitions 0-63, odd on 64-127, so the two K=64 score matmuls run
    CONCURRENTLY in disjoint PE row groups (tile_position auto-derived).
  - q_projT is duplicated onto both partition halves via a second matmul
    with tile_position=(0, 64) (compute engines cannot shift partitions).
  - exp covers an even+odd pair in one ACTIVATE (FD=1024 from 2 psum banks).
  - all matmul operands bf16 (PSUM accumulation stays f32).
"""

import os
import sys

import numpy as np

if "/opt/trn_rl_repo" not in sys.path:
    sys.path.insert(0, "/opt/trn_rl_repo")

S, B, D, C = 4096, 4, 128, 64
NCORES = 8
QS = S // NCORES  # 512 queries per core
QT = QS // 128  # 4 q tiles
ST = S // 128  # 32 s tiles
NP = ST // 2  # 16 even/odd s-tile pairs
SCALE = 0.125  # 1/sqrt(64)

LAST_RESULT = None


def _install_ntff_hook():
    """The grading/axon image lacks antenv.axon_hooks; recreate it so
    trace=True can capture NTFF profiles. Harmless no-op when unavailable."""
    import types

    try:
        import antenv

        try:
            from antenv import axon_hooks  # noqa: F401

            return
        except ImportError:
            pass
        from trn_agent_boot.trn_boot import _ntff_profile_via_ctypes

        mod = types.ModuleType("antenv.axon_hooks")
        _h = [_ntff_profile_via_ctypes("/opt/axon/libaxon_pjrt.so")]
        mod.get_axon_ntff_profile_hook = lambda: _h[0]
        mod.set_axon_ntff_profile_hook = lambda h: _h.__setitem__(0, h)
        sys.modules["antenv.axon_hooks"] = mod
        antenv.axon_hooks = mod
    except Exception:
        pass


def _build_nc():
    import concourse.mybir as mybir
    from concourse import bacc
    from concourse.masks import make_identity
    from concourse.tile import TileContext

    f32 = mybir.dt.float32
    bf16 = mybir.dt.bfloat16
    i32 = mybir.dt.int32
    AF = mybir.ActivationFunctionType

    nc = bacc.Bacc("TRN2")

    key_d = nc.dram_tensor("key", [S, B, D], f32, kind="ExternalInput")
    query_d = nc.dram_tensor("query", [QS, B, D], f32, kind="ExternalInput")
    value_d = nc.dram_tensor("value", [S, B, D], f32, kind="ExternalInput")
    mask_d = nc.dram_tensor("mask", [QS, S], i32, kind="ExternalInput")
    wk_d = nc.dram_tensor("wk_w", [C, D], f32, kind="ExternalInput")
    wq_d = nc.dram_tensor("wq_w", [C, D], f32, kind="ExternalInput")
    wv_d = nc.dram_tensor("wv_w", [C, D], f32, kind="ExternalInput")
    bk_d = nc.dram_tensor("wk_b", [C], f32, kind="ExternalInput")
    bq_d = nc.dram_tensor("wq_b", [C], f32, kind="ExternalInput")
    bv_d = nc.dram_tensor("wv_b", [C], f32, kind="ExternalInput")
    out_d = nc.dram_tensor("out", [QS, B, C], f32, kind="ExternalOutput")

    with TileContext(nc) as tc:
        with (
            tc.tile_pool(name="consts", bufs=1) as consts,
            tc.tile_pool(name="big", bufs=1) as big,
            tc.tile_pool(name="pb", bufs=2) as pb,
            tc.tile_pool(name="work", bufs=4) as work,
            tc.tile_pool(name="apool", bufs=2) as apool,
            tc.tile_pool(name="scps", bufs=2, space="PSUM") as scps,
            tc.tile_pool(name="accps", bufs=1, space="PSUM") as accps,
            tc.tile_pool(name="pps", bufs=2, space="PSUM") as pps,
            tc.tile_pool(name="dram", bufs=1, space="DRAM") as dram,
        ):
            # ---------------- constants ----------------
            ident_f = consts.tile([128, 128], f32, tag="ident_f")
            make_identity(nc, ident_f[:])
            ones_b = consts.tile([128, 1], bf16, tag="ones_b")
            nc.vector.memset(ones_b[:], 1.0)

            wk_sb = consts.tile([C, D], f32, tag="wk_sb")
            nc.sync.dma_start(out=wk_sb[:], in_=wk_d[:, :])
            wq_sb = consts.tile([C, D], f32, tag="wq_sb")
            nc.sync.dma_start(out=wq_sb[:], in_=wq_d[:, :])
            wv_sb = consts.tile([C, D], f32, tag="wv_sb")
            nc.sync.dma_start(out=wv_sb[:], in_=wv_d[:, :])

            # biases replicated on both partition halves [128, 1]
            bk2 = consts.tile([128, 1], f32, tag="bk2")
            bq2 = consts.tile([128, 1], f32, tag="bq2")
            for half in (slice(0, 64), slice(64, 128)):
                nc.sync.dma_start(
                    out=bk2[half, :], in_=bk_d[:].rearrange("(c one) -> c one", one=1)
                )
                nc.sync.dma_start(
                    out=bq2[half, :], in_=bq_d[:].rearrange("(c one) -> c one", one=1)
                )
            bv_row = consts.tile([1, C], bf16, tag="bv_row")
            bv_f = consts.tile([1, C], f32, tag="bv_f")
            nc.sync.dma_start(
                out=bv_f[:], in_=bv_d[:].rearrange("(one c) -> one c", one=1)
            )
            nc.vector.tensor_copy(out=bv_row[:], in_=bv_f[:])

            # transposed weights [D, C] bf16 via PE transpose
            wT = {}
            for name, w_sb in (("k", wk_sb), ("q", wq_sb), ("v", wv_sb)):
                wt_ps = pps.tile([D, C], f32, tag="pps", name=f"wt_ps_{name}")
                nc.tensor.transpose(wt_ps[:], w_sb[:], ident_f[:C, :C])
                wt_sb = consts.tile([D, C], bf16, name=f"wt_sb_{name}")
                nc.vector.tensor_copy(out=wt_sb[:], in_=wt_ps[:])
                wT[name] = wt_sb

            # ---------------- mask transpose (batch-shared) ----------------
            # cast i32->bf16 on DVE, stage to DRAM, then xbar DMA transposes
            # maskT[s, q] = mask[q, s] as bf16, laid out [128, (st, q)]
            maskT = big.tile([128, ST * QS], bf16, tag="maskT")
            mscr = dram.tile([QS, S], bf16, tag="mscr")
            for qt in range(QT):
                for g in range(8):
                    m_i = work.tile([128, 512], i32, tag="m_i")
                    nc.sync.dma_start(
                        out=m_i[:],
                        in_=mask_d[qt * 128 : (qt + 1) * 128, g * 512 : (g + 1) * 512],
                    )
                    m_b = work.tile([128, 512], bf16, tag="m_b")
                    nc.vector.tensor_copy(out=m_b[:], in_=m_i[:])
                    nc.sync.dma_start(
                        out=mscr[qt * 128 : (qt + 1) * 128, g * 512 : (g + 1) * 512],
                        in_=m_b[:],
                    )
            maskT_v = maskT[:].rearrange("p (st q) -> p st q", st=ST)
            for st in range(ST):
                nc.sync.dma_start_transpose(
                    maskT_v[:, st, :],
                    mscr[:, st * 128 : (st + 1) * 128],
                )

            # ---------------- per batch ----------------
            kscr = dram.tile([B, S, D], bf16, tag="kscr")
            for b in range(B):
                # key^T [d, s] bf16 via DRAM-staged xbar DMA transposes
                keyT = pb.tile([128, S], bf16, tag="keyT")
                for g in range(4):
                    k_nat = work.tile([128, 1024], f32, tag="k_nat")
                    nc.sync.dma_start(
                        out=k_nat[:].rearrange("p (t d) -> p t d", t=8),
                        in_=key_d[g * 1024 : (g + 1) * 1024, b, :].rearrange(
                            "(t p) d -> p t d", p=128
                        ),
                    )
                    k_bf = work.tile([128, 1024], bf16, tag="k_bf")
                    nc.scalar.copy(out=k_bf[:], in_=k_nat[:])
                    nc.sync.dma_start(
                        out=kscr[b, g * 1024 : (g + 1) * 1024, :].rearrange(
                            "(t p) d -> p t d", p=128
                        ),
                        in_=k_bf[:].rearrange("p (t d) -> p t d", t=8),
                    )
                for sr in range(8):
                    nc.sync.dma_start_transpose(
                        keyT[:, sr * 512 : (sr + 1) * 512],
                        kscr[b, sr * 512 : (sr + 1) * 512, :],
                    )

                # k_projT2: even s-tiles on partitions 0-63, odd on 64-127.
                # [128, NP*128] bf16; pair u occupies cols [u*128, (u+1)*128)
                k_projT2 = pb.tile([128, NP * 128], bf16, tag="k_projT2")
                keyT_v = keyT[:].rearrange(
                    "d (c bb two j) -> d c bb two j", c=4, bb=4, two=2
                )
                # col of keyT = st*128 + j, st = 8c + 2*bb + two
                for c in range(4):
                    kp_ps = pps.tile([128, 512], f32, tag="pps", name="kp_ps")
                    nc.tensor.matmul(
                        kp_ps[:64, :],
                        wT["k"][:],
                        keyT_v[:, c, :, 0, :],
                        start=True,
                        stop=True,
                    )
                    nc.tensor.matmul(
                        kp_ps[64:, :],
                        wT["k"][:],
                        keyT_v[:, c, :, 1, :],
                        start=True,
                        stop=True,
                        tile_position=(0, 64),
                    )
                    nc.vector.tensor_scalar_add(
                        out=k_projT2[:, c * 512 : (c + 1) * 512],
                        in0=kp_ps[:],
                        scalar1=bk2[:],
                    )

                # q_projT3 [128, 512] bf16 (same data on both halves)
                q_nat = work.tile([128, 512], f32, tag="q_nat")
                nc.sync.dma_start(
                    out=q_nat[:].rearrange("p (t d) -> p t d", t=4),
                    in_=query_d[:, b, :].rearrange("(t p) d -> p t d", p=128),
                )
                qt_ps = pps.tile([128, 512], f32, tag="pps", name="qt_ps")
                for i in range(4):
                    nc.tensor.transpose(
                        qt_ps[:, i * 128 : (i + 1) * 128],
                        q_nat[:, i * 128 : (i + 1) * 128],
                        ident_f[:],
                    )
                qT = work.tile([128, 512], bf16, tag="qT")
                nc.vector.tensor_copy(out=qT[:], in_=qt_ps[:])
                qp_ps = pps.tile([128, 512], f32, tag="pps", name="qp_ps")
                nc.tensor.matmul(qp_ps[:64, :], wT["q"][:], qT[:], start=True, stop=True)
                nc.tensor.matmul(
                    qp_ps[64:, :],
                    wT["q"][:],
                    qT[:],
                    start=True,
                    stop=True,
                    tile_position=(0, 64),
                )
                q_projT3 = pb.tile([128, QS], bf16, tag="q_projT3")
                nc.vector.tensor_scalar_add(
                    out=q_projT3[:],
                    in0=qp_ps[:],
                    scalar1=bq2[:],
                )

                # value natural [s, d] -> bf16 (gpsimd casts; 1-input = cheap)
                v_f32 = pb.tile([128, S], f32, tag="v_f32")
                for g in range(8):
                    nc.sync.dma_start(
                        out=v_f32[:, g * 512 : (g + 1) * 512].rearrange(
                            "p (t d) -> p t d", t=4
                        ),
                        in_=value_d[g * 512 : (g + 1) * 512, b, :].rearrange(
                            "(t p) d -> p t d", p=128
                        ),
                    )
                v_sb = pb.tile([128, S], bf16, tag="v_sb")
                for g in range(2):
                    nc.vector.tensor_copy(
                        out=v_sb[:, g * 2048 : (g + 1) * 2048],
                        in_=v_f32[:, g * 2048 : (g + 1) * 2048],
                    )

                # ---------------- main loop over s-tile pairs ----------------
                va_ps = accps.tile([128, QS], f32, tag="va")
                sums_ps = accps.tile([1, QS], f32, tag="sums", bufs=1)
                for u in range(NP):
                    sc_ps = scps.tile([128, 1024], f32, tag="sc")
                    nc.tensor.matmul(
                        sc_ps[:, :512],
                        k_projT2[:64, u * 128 : (u + 1) * 128],
                        q_projT3[:64, :],
                        start=True,
                        stop=True,
                    )
                    nc.tensor.matmul(
                        sc_ps[:, 512:],
                        k_projT2[64:, u * 128 : (u + 1) * 128],
                        q_projT3[64:, :],
                        start=True,
                        stop=True,
                    )
                    ex = apool.tile([128, 1024], bf16, tag="ex")
                    nc.scalar.activation(
                        out=ex[:], in_=sc_ps[:], func=AF.Exp, scale=SCALE
                    )
                    alpha = apool.tile([128, 1024], bf16, tag="alpha")
                    nc.vector.tensor_mul(
                        alpha[:], ex[:], maskT[:, u * 1024 : (u + 1) * 1024]
                    )
                    nc.tensor.matmul(
                        va_ps[:],
                        v_sb[:, (2 * u) * 128 : (2 * u + 1) * 128],
                        alpha[:, :512],
                        start=(u == 0),
                        stop=False,
                    )
                    nc.tensor.matmul(
                        va_ps[:],
                        v_sb[:, (2 * u + 1) * 128 : (2 * u + 2) * 128],
                        alpha[:, 512:],
                        start=False,
                        stop=(u == NP - 1),
                    )
                    nc.tensor.matmul(
                        sums_ps[:],
                        ones_b[:],
                        alpha[:, :512],
                        start=(u == 0),
                        stop=False,
                    )
                    nc.tensor.matmul(
                        sums_ps[:],
                        ones_b[:],
                        alpha[:, 512:],
                        start=False,
                        stop=(u == NP - 1),
                    )

                # ---------------- epilogue ----------------
                va_sb = work.tile([128, QS], bf16, tag="va_sb")
                nc.scalar.copy(out=va_sb[:], in_=va_ps[:])
                sums_b = work.tile([1, QS], bf16, tag="sums_b")
                nc.scalar.copy(out=sums_b[:], in_=sums_ps[:])

                outT_ps = pps.tile([C, QS], f32, tag="pps", name="outT_ps")
                nc.tensor.matmul(
                    outT_ps[:], wT["v"][:], va_sb[:], start=True, stop=False
                )
                nc.tensor.matmul(
                    outT_ps[:], bv_row[:], sums_b[:], start=False, stop=True
                )

                comb = work.tile([C + 1, QS], f32, tag="comb")
                nc.scalar.copy(out=comb[:C, :], in_=outT_ps[:])
                nc.scalar.copy(out=comb[C : C + 1, :], in_=sums_ps[:])

                for qt in range(QT):
                    ot_ps = pps.tile([128, C + 1], f32, tag="pps", name="ot_ps")
                    nc.tensor.transpose(
                        ot_ps[:],
                        comb[:, qt * 128 : (qt + 1) * 128],
                        ident_f[: C + 1, : C + 1],
                    )
                    o_nat = work.tile([128, C + 1], f32, tag="o_nat")
                    nc.scalar.copy(out=o_nat[:], in_=ot_ps[:])
                    recip = work.tile([128, 1], f32, tag="recip")
                    nc.vector.reciprocal(recip[:], o_nat[:, C : C + 1])
                    final = work.tile([128, C], f32, tag="final")
                    nc.scalar.activation(
                        out=final[:], in_=o_nat[:, :C], func=AF.Copy, scale=recip[:]
                    )
                    nc.sync.dma_start(
                        out=out_d[qt * 128 : (qt + 1) * 128, b, :], in_=final[:]
                    )

    nc.finalize()
    return nc


_nc_cache = None


def kernel(**inputs):
    global _nc_cache, LAST_RESULT
    _install_ntff_hook()
    from concourse.bass_utils import run_bass_kernel_spmd

    arrs = {k: np.asarray(v) for k, v in inputs.items()}
    key = np.ascontiguousarray(arrs["key"], dtype=np.float32)
    query = np.ascontiguousarray(arrs["query"], dtype=np.float32)
    value = np.ascontiguousarray(arrs["value"], dtype=np.float32)
    mask = np.ascontiguousarray(arrs["mask"], dtype=np.int32)
    if mask.ndim == 3:
        mask = mask[0]

    if _nc_cache is None:
        _nc_cache = _build_nc()
    nc = _nc_cache

    in_maps = []
    for i in range(NCORES):
        q0 = i * QS
        in_maps.append(
            {
                "key": key,
                "value": value,
                "query": np.ascontiguousarray(query[q0 : q0 + QS]),
                "mask": np.ascontiguousarray(mask[q0 : q0 + QS]),
                "wk_w": np.ascontiguousarray(arrs["wk_w"], dtype=np.float32),
                "wq_w": np.ascontiguousarray(arrs["wq_w"], dtype=np.float32),
                "wv_w": np.ascontiguousarray(arrs["wv_w"], dtype=np.float32),
                "wk_b": np.ascontiguousarray(arrs["wk_b"], dtype=np.float32),
                "wq_b": np.ascontiguousarray(arrs["wq_b"], dtype=np.float32),
                "wv_b": np.ascontiguousarray(arrs["wv_b"], dtype=np.float32),
            }
        )

    trace = bool(int(os.environ.get("KERNEL_TRACE", "0")))
    kw = {}
    if trace:
        kw = dict(trace=True, trace_cores=[0])
    res = run_bass_kernel_spmd(nc, in_maps, core_ids=list(range(NCORES)), **kw)
    LAST_RESULT = res
    out = np.concatenate([r["out"] for r in res.results], axis=0)
    return out
